# revision 47
# baseline (speedup 1.0000x reference)
"""LIF spiking-neuron scan on 8 Trainium2 NeuronCores (~53.5us, exact).

Pipeline per core:
  DVE   fused LIF recurrence (custom multi-step op, 60-step mem ring)
  ACT   sign codes sign(mem - b) in {-1,0,+1} as fp8, steps 0..88
  PE    packs 4 codes/output with base-4 weights (S = sum_j c_j 4^j,
        |S| <= 85, uniquely decodable) -- 4 matmuls per PSUM bank tile
        [128,512] at tile_position cols 0/32/64/96
  ACT   drains each PSUM tile to int8 in SBUF (0.25B per spike)
  steps 88..100 skip all of that: raw mem f32 is stored straight from
        the ring per scan block, host does the exact mem>b compare --
        the kernel tail is scan->store, no sign/PE/drain latency.

HBM traffic per core: 13.1MB input + ~0.57MB packed + 1.57MB raw tail.
Loads ride the SP + ACT HWDGE queues at ~10-step granularity (one queue
alone tops out at ~325GB/s, two reach ~425; DMA-completion semaphores
lag data by several us, so only the last small chunk's lag is exposed).
Consts load via the quiet gpsimd SWDGE queue during the preamble; bulk
data must NOT use it (SWDGE descriptor generation stalls everything).
Timeline: ~6.6us fixed preamble, loads visible by ~43.5us, scan tail +
pack tail both land ~50-51.5us, ~7us fixed semaphore-slot teardown.

Sharding: batch dim (64) split 8 ways -> per core [T=100, 8, 4096] =
[128 partitions, 100*256 f32] time-major; whole input persists in SBUF.
"""

import os

import numpy as np

import concourse.bass as bass
import concourse.mybir as mybir
from concourse.bass_utils import run_bass_kernel_spmd

T, B, N = 100, 64, 4096
NCORES = 8
P = 128
BPC = B // NCORES          # batch rows per core
FD = BPC * N // P          # 256 free elems per partition per timestep
F32 = mybir.dt.float32
F8 = mybir.dt.float8e4
I8 = mybir.dt.int8

EPS = np.float32(1e-8)

# DVE scan blocks (custom op validated exact up to 12 steps; 10 used).
# 80-90 is split 5+5 so the packed pipeline's last sign blocks are short.
BLOCKS = [2, 3, 5] + [10] * 7 + [5, 5, 4, 3, 2, 1]
assert sum(BLOCKS) == T
STARTS = [sum(BLOCKS[:i]) for i in range(len(BLOCKS))]
ENDS = [STARTS[i] + BLOCKS[i] for i in range(len(BLOCKS))]
NBL = len(BLOCKS)
NSGN = 12                  # sign blocks 0..11 cover steps 0..90
MRING = 60                 # mem ring length in steps

# input load chunks = DVE blocks (fine granularity keeps the DVE overlapped
# with the load tail; DMA-completion sem updates lag data by several us, so
# only the last small chunk's lag lands on the critical path)
LCH = list(BLOCKS)
LCUM = [sum(LCH[:i + 1]) for i in range(len(LCH))]
NLC = len(LCH)
SP_CH = list(range(0, NLC, 2))     # chunks issued on the SP queue
AC_CH = list(range(1, NLC, 2))     # chunks issued on the ACT queue

# pack geometry: pair = 2 steps (512 cols), tile = 4 pairs -> PSUM [128,512].
# Steps 88..100 skip the pack path entirely: raw mem f32 is stored straight
# from the ring per scan block (gated on DVE progress only), and the host
# does the exact mem>b compare.  The kernel tail is then scan->store.
TRAW = 88
NPAIR = TRAW // 2                  # 44 packed pairs
NTILE = NPAIR // 4                 # 11 tiles
PKW = NTILE * 512                  # packed out cols (i8)

# packed store groups on SP (the final tile 10 is stored by ACT right after
# its own drain -- same-engine ordering, no semaphore, empty queue)
STG = [(0, 4), (4, 8), (8, 10)]

# tail raw-mem stores on SP: (start step, n steps, mrd gate).  Few-ish
# stores: every completion costs 16 notification events, and bunched
# events near the kernel end inflate the final semaphore lag.
TAILMS = [(88, 6, 13), (94, 4, 15), (98, 2, 16)]

_TRACE = bool(os.environ.get("LIF_TRACE"))
LAST_RUN = None  # BassKernelResults of the most recent run (for test.py)


# ---------------------------------------------------------------------------
# Custom fused DVE op registration (runtime-append to concourse.dve_ops.OPS)
# ---------------------------------------------------------------------------
_REGISTERED = {}


def _register_lif_op(ge_mask: bool):
    """LIF_STEP_(LE|GE): out = (Src0 cmp C0) * Src0 * C1 + Src1."""
    name = "LIF_STEP_GE" if ge_mask else "LIF_STEP_LE"
    if name in _REGISTERED:
        return _REGISTERED[name]
    import concourse.dve_ops as dops
    from concourse.dve_spec import Spec, Src0, Src1, C0, C1, lower
    from concourse.dve_uop import DveOpSpec

    if ge_mask:
        body = (Src0 >= C0) * Src0 * C1 + Src1
        ref = lambda in0, in1, s0, s1, imm2: (
            (in0 * (in0 >= s0).astype(np.float32)).astype(np.float32)
            * np.float32(s1) + in1).astype(np.float32)
    else:
        body = (Src0 <= C0) * Src0 * C1 + Src1
        ref = lambda in0, in1, s0, s1, imm2: (
            (in0 * (in0 <= s0).astype(np.float32)).astype(np.float32)
            * np.float32(s1) + in1).astype(np.float32)
    spec = Spec(body=body, reference=ref)

    row = dops._CUSTOM_DVE_ROW_BASE + len(dops.OPS)
    assert row < 0x20, "custom-DVE opcode rows exhausted"
    shas = {}
    for ver in ("v3", "v4"):
        shas[ver] = DveOpSpec(
            name=name, opcode=row, uops=lower(spec, ver=ver), rd1_en=True
        ).sha(ver)
    op = dops.DveOp(name, spec, subdim=False, uops_sha=shas)
    dops.OPS.append(op)
    dops.CUSTOM_DVE_SPECS[name] = spec
    dops._SUB_OPCODE_FOR_NAME[name] = row
    _REGISTERED[name] = op
    return op


# ---------------------------------------------------------------------------
# Threshold boundary (host-side, exact)
# ---------------------------------------------------------------------------
def _predicate(vth: np.float32):
    c = np.float32(vth + EPS)
    assert c != 0.0, "degenerate threshold"
    one = np.float32(1.0)
    if vth > 0:
        pred = lambda m: (np.float32(np.float32(m) / c) - one) > 0
        increasing = True
    else:
        pred = lambda m: (one - np.float32(np.float32(m) / c)) > 0
        increasing = c < 0
    return pred, increasing


def _f32_key(m) -> int:
    i = int(np.frombuffer(np.float32(m).tobytes(), np.uint32)[0])
    return i ^ 0xFFFFFFFF if i & 0x80000000 else i | 0x80000000


def _key_f32(k: int):
    u = (k & 0x7FFFFFFF) if k & 0x80000000 else (k ^ 0xFFFFFFFF)
    return np.frombuffer(np.uint32(u).tobytes(), np.float32)[0]


def spike_boundary(vth: np.float32):
    """Exact f32 boundary b of the spike predicate.
    spk_is_gt: spk = (mem > b), device no-spike mask = (mem is_le b)."""
    with np.errstate(over="ignore"):
        pred, increasing = _predicate(vth)
        lo_k, hi_k = _f32_key(np.float32(-3.4e38)), _f32_key(np.float32(3.4e38))
        if increasing:
            assert not pred(_key_f32(lo_k)) and pred(_key_f32(hi_k))
            while hi_k - lo_k > 1:
                mid = (lo_k + hi_k) // 2
                if pred(_key_f32(mid)):
                    hi_k = mid
                else:
                    lo_k = mid
            b = _key_f32(lo_k)
            assert not pred(b) and pred(_key_f32(lo_k + 1))
            return b, True
        else:
            assert pred(_key_f32(lo_k)) and not pred(_key_f32(hi_k))
            while hi_k - lo_k > 1:
                mid = (lo_k + hi_k) // 2
                if pred(_key_f32(mid)):
                    lo_k = mid
                else:
                    hi_k = mid
            b = _key_f32(hi_k)
            assert not pred(b) and pred(_key_f32(hi_k - 1))
            return b, False


# ---------------------------------------------------------------------------
# Static schedule helpers
# ---------------------------------------------------------------------------
def _sign_block_for_step(s: int) -> int:
    return next(j for j in range(NBL) if STARTS[j] <= s < ENDS[j])


def _tile_gate_block(t: int) -> int:
    # tile t needs steps < min(8(t+1), TRAW) signed
    need = min(8 * (t + 1), TRAW)
    return next(j for j in range(NBL) if ENDS[j] >= need)


def _act_oplist():
    """ACT program order: ("sign", b) and ("drain", t) interleaved.
    Drain t placed after sign block min(gate(t)+1, NSGN-1)."""
    place = {}
    for t in range(NTILE):
        pb = min(_tile_gate_block(t) + 1, NSGN - 1)
        if t in (8, 9):
            # keep the last sign (gates the final PE tile) ahead of these
            pb = NSGN - 1
        place.setdefault(pb, []).append(t)
    ops = []
    for b in range(NSGN):
        ops.append(("sign", b))
        for t in place.get(b, []):
            ops.append(("drain", t))
    assert len(ops) == NSGN + NTILE
    return ops


ACTOPS = _act_oplist()
APSIDX = {op: i + 1 for i, op in enumerate(ACTOPS)}  # aps value when op done


# ---------------------------------------------------------------------------
# Device program
# ---------------------------------------------------------------------------
def build_program(c2: float, d: float, spk_is_gt: bool) -> bass.Bass:
    lif_op = _register_lif_op(ge_mask=not spk_is_gt)
    nc = bass.Bass("TRN2", target_bir_lowering=False, debug=False,
                   enable_asserts=False)
    x_d = nc.dram_tensor("x", [P, T * FD], F32, kind="ExternalInput")
    w_d = nc.dram_tensor("w", [P, 32], F32, kind="ExternalInput")
    nb_d = nc.dram_tensor("nbias", [P, 1], F32, kind="ExternalInput")
    y_d = nc.dram_tensor("y", [P, PKW], I8, kind="ExternalOutput")
    yr_d = nc.dram_tensor("yr", [P, (T - TRAW) * FD], F32, kind="ExternalOutput")

    xb = nc.alloc_sbuf_tensor("xb", [P, T * FD], F32)        # whole input
    mb = nc.alloc_sbuf_tensor("mb", [P, MRING * FD], F32)    # mem ring
    code = nc.alloc_sbuf_tensor("code", [P, T * FD], F8)     # sign codes
    pko = nc.alloc_sbuf_tensor("pko", [P, PKW], I8)          # packed out
    wf = nc.alloc_sbuf_tensor("wf", [P, 32], F32)
    w8 = nc.alloc_sbuf_tensor("w8", [P, 32], F8)
    cst = nc.alloc_sbuf_tensor("const-lif-bias", [P, 1], F32)
    nc.const_aps.aps[(F32, -float(c2))] = cst.ap()

    pk = [nc.alloc_psum_tensor(f"pk{i}", [P, 512], F32) for i in range(8)]

    xc = [nc.alloc_semaphore(f"xc{c}") for c in range(NLC)]
    mrd = nc.alloc_semaphore("mrd")   # DVE block progress
    aps = nc.alloc_semaphore("aps")   # ACT op progress (signs + drains)
    pem = nc.alloc_semaphore("pem")   # PE tile progress
    cbs = nc.alloc_semaphore("cbs")   # sign-bias const loaded
    gpr = nc.alloc_semaphore("gpr")   # weights loaded / converted
    stm = nc.alloc_semaphore("stm")   # store completions

    def xdsl(c):
        lo = (LCUM[c] - LCH[c]) * FD
        return x_d[:, lo:lo + LCH[c] * FD]

    def xsl(c):
        lo = (LCUM[c] - LCH[c]) * FD
        return xb[:, lo:lo + LCH[c] * FD]

    def msl(s0, n=1):
        # steps [s0, s0+n) in ring coords; caller guarantees no wrap
        p = (s0 % MRING) * FD
        assert (s0 % MRING) + n <= MRING, (s0, n)
        return mb[:, p:p + n * FD]

    # mem-ring reuse: DVE block b overwrites ring cells of steps
    # start..end-1 minus MRING -> need those signed (aps threshold)
    def reuse_gate(b):
        last_old = ENDS[b] - 1 - MRING
        if last_old < 0:
            return None
        return APSIDX[("sign", _sign_block_for_step(last_old))]

    with nc.Block() as blk:

        @blk.sync
        def _(sync):
            for c in SP_CH:
                sync.dma_start(xsl(c), xdsl(c)).then_inc(xc[c], 16)
            for t0, t1 in STG[:2]:
                sync.wait_ge(aps, APSIDX[("drain", t1 - 1)])
                sync.dma_start(y_d[:, 512 * t0:512 * t1],
                               pko[:, 512 * t0:512 * t1]).then_inc(stm, 16)
            # tail raw-mem stores, gated on DVE progress only
            for s0, n, g in TAILMS:
                sync.wait_ge(mrd, g)
                sync.dma_start(yr_d[:, (s0 - TRAW) * FD:(s0 - TRAW + n) * FD],
                               msl(s0, n)).then_inc(stm, 16)
            t0, t1 = STG[-1]
            sync.wait_ge(aps, APSIDX[("drain", t1 - 1)])
            sync.dma_start(y_d[:, 512 * t0:512 * t1],
                           pko[:, 512 * t0:512 * t1]).then_inc(stm, 16)
            sync.wait_ge(stm, 16 * (len(STG) + len(TAILMS) + 1))

        @blk.gpsimd
        def _(gp):
            # tiny consts on the quiet gpsimd queue (complete in preamble)
            gp.dma_start(cst[:, :], nb_d[:, :]).then_inc(cbs, 16)
            gp.dma_start(wf[:, :], w_d[:, :]).then_inc(gpr, 16)

        @blk.scalar
        def _(act):
            for c in AC_CH:
                act.dma_start(xsl(c), xdsl(c)).then_inc(xc[c], 16)
            act.wait_ge(cbs, 16)             # sign-bias const loaded
            first = True
            for kind, i in ACTOPS:
                if kind == "sign":
                    b = i
                    s0, K = STARTS[b], BLOCKS[b]
                    if s0 + K > TRAW:
                        # steps >= TRAW ship as raw mem; don't sign them
                        K = TRAW - s0
                    ins = act.activation(
                        code[:, s0 * FD:(s0 + K) * FD],
                        msl(s0, K),
                        mybir.ActivationFunctionType.Sign,
                        bias=-float(c2), scale=1.0,
                    )
                    ins._wait_ge(mrd, b + 1)
                else:
                    t = i
                    ins = act.copy(pko[:, 512 * t:512 * (t + 1)],
                                   pk[t % 8][:, :])
                    ins._wait_ge(pem, t + 1)
                ins.then_inc(aps, 1)
                if first:
                    # weight fp8 convert tucked behind sign block 0
                    first = False
                    cv = act.copy(w8[:, :], wf[:, :])
                    cv._wait_ge(gpr, 16)
                    cv.then_inc(gpr, 1)
            # tile 10 store straight after its drain (same engine, in order)
            act.dma_start(y_d[:, 512 * (NTILE - 1):512 * NTILE],
                          pko[:, 512 * (NTILE - 1):512 * NTILE]
                          ).then_inc(stm, 16)

        @blk.vector
        def _(v):
            for b in range(NBL):
                g = reuse_gate(b)
                if g is not None:
                    v.wait_ge(aps, g)
                s0, K = STARTS[b], BLOCKS[b]
                cb = b
                if b == 0:
                    ins = v.tensor_copy(msl(0), xb[:, 0:FD])
                    ins._wait_ge(xc[0], 16)
                    last = v._custom_dve(
                        lif_op, out=msl(1), in0=msl(0),
                        in1=xb[:, FD:2 * FD], s0=float(c2), s1=float(d))
                elif (s0 - 1) % MRING + K > MRING:
                    # ring wrap between in0 start and out end: stitch 1 step
                    ins = v._custom_dve(
                        lif_op, out=msl(s0), in0=msl(s0 - 1),
                        in1=xb[:, s0 * FD:(s0 + 1) * FD],
                        s0=float(c2), s1=float(d))
                    ins._wait_ge(xc[cb], 16)
                    last = ins
                    if K > 1:
                        last = v._custom_dve(
                            lif_op, out=msl(s0 + 1, K - 1),
                            in0=msl(s0, K - 1),
                            in1=xb[:, (s0 + 1) * FD:(s0 + K) * FD],
                            s0=float(c2), s1=float(d))
                else:
                    last = v._custom_dve(
                        lif_op, out=msl(s0, K), in0=msl(s0 - 1, K),
                        in1=xb[:, s0 * FD:(s0 + K) * FD],
                        s0=float(c2), s1=float(d))
                    last._wait_ge(xc[cb], 16)
                last.then_inc(mrd, 1)

        @blk.tensor
        def _(pe):
            pe.wait_ge(gpr, 17)              # w8 converted
            for t in range(NTILE):
                if t == NTILE - 1:
                    # first half (pairs 40,41 = steps 80..84): sign block 10
                    gb = next(j for j in range(NBL) if ENDS[j] >= 8 * t + 4)
                else:
                    gb = _tile_gate_block(t)
                thr = APSIDX[("sign", gb)]
                if t >= 8:
                    thr = max(thr, APSIDX[("drain", t - 8)])
                pe.wait_ge(aps, thr)
                for m in range(4):
                    q = 4 * t + m
                    if t == NTILE - 1 and m == 2:
                        # pairs 42,43 (steps 84..88) need sign block 11
                        pe.wait_ge(aps, APSIDX[("sign", NSGN - 1)])
                    ins = pe.matmul(
                        out=pk[t % 8][32 * m:32 * (m + 1), :],
                        lhsT=w8[:, :],
                        rhs=code[:, 512 * q:512 * (q + 1)],
                        start=True, stop=True,
                        tile_position=(0, 32 * m),
                    )
                ins.then_inc(pem, 1)

    mybir.codegen_inst_isa_subclasses(nc)
    return nc


_PROGRAM_CACHE: dict = {}


def _pack_weights() -> np.ndarray:
    w = np.zeros((P, 32), dtype=np.float32)
    for p in range(P):
        w[p, p // 4] = 4.0 ** (p % 4)
    return w


def _decode_core(y: np.ndarray, yr: np.ndarray, bnd: np.float32,
                 spk_is_gt: bool) -> np.ndarray:
    """y: [128, NTILE*512] int8 packed (steps 0..TRAW),
    yr: [128, (T-TRAW)*256] f32 raw mem -> spikes [T, 128, 256] bool."""
    # rows: 128 = 4 pair-slots (m) x 32 row-groups (r)
    # cols: NTILE*512 = NTILE tiles (t) x 2 step-offsets (so) x 256 elems (e)
    S = y.reshape(4, 32, NTILE, 2, 256).astype(np.int32)
    target = 1 if spk_is_gt else -1
    sp = np.empty((T, 128, 256), dtype=bool)
    pk = sp[:TRAW].reshape(NTILE, 4, 2, 32, 4, 256)
    for j in range(4):
        r = np.mod(S, 4)
        cj = np.where(r == 3, -1, r)
        # axes (m, r, t, so, e) -> (t, m, so, r, e)
        pk[:, :, :, :, j, :] = (cj == target).transpose(2, 0, 3, 1, 4)
        S = (S - cj) // 4
    raw = yr.reshape(128, T - TRAW, 256)
    cmp = (raw > bnd) if spk_is_gt else (raw < bnd)
    sp[TRAW:] = cmp.transpose(1, 0, 2)
    return sp


def kernel(inpt: np.ndarray, v_th: np.ndarray, v_decay: np.ndarray) -> np.ndarray:
    global LAST_RUN
    x = np.ascontiguousarray(np.asarray(inpt, dtype=np.float32))
    assert x.shape == (T, B, N), x.shape
    vth = np.float32(np.asarray(v_th))
    d = float(np.float32(np.asarray(v_decay)))
    b, spk_is_gt = spike_boundary(vth)

    key = (float(b), d, spk_is_gt)
    if key not in _PROGRAM_CACHE:
        _PROGRAM_CACHE[key] = build_program(float(b), d, spk_is_gt)
    nc = _PROGRAM_CACHE[key]

    in_maps = []
    nbias = np.full((P, 1), -np.float32(b), dtype=np.float32)
    wpack = _pack_weights()
    for k in range(NCORES):
        xk = x[:, k * BPC:(k + 1) * BPC, :].reshape(T, P, FD)
        xk = np.ascontiguousarray(xk.transpose(1, 0, 2)).reshape(P, T * FD)
        in_maps.append({"x": xk, "nbias": nbias, "w": wpack})

    res = run_bass_kernel_spmd(
        nc, in_maps, core_ids=list(range(NCORES)), trace=_TRACE
    )
    LAST_RUN = res

    spikes = np.empty((T, B, N), dtype=np.float32)
    for k in range(NCORES):
        sp = _decode_core(res.results[k]["y"], res.results[k]["yr"],
                          np.float32(b), spk_is_gt)
        spikes[:, k * BPC:(k + 1) * BPC, :] = sp.reshape(T, BPC, N)
    return spikes


# revision 48
# speedup vs baseline: 1.1753x; 1.1753x over previous
"""LIF spiking-neuron scan on 8 Trainium2 NeuronCores (~53.5us, exact).

Pipeline per core:
  DVE   fused LIF recurrence (custom multi-step op, 60-step mem ring)
  ACT   sign codes sign(mem - b) in {-1,0,+1} as fp8, steps 0..88
  PE    packs 4 codes/output with base-4 weights (S = sum_j c_j 4^j,
        |S| <= 85, uniquely decodable) -- 4 matmuls per PSUM bank tile
        [128,512] at tile_position cols 0/32/64/96
  ACT   drains each PSUM tile to int8 in SBUF (0.25B per spike)
  steps 88..100 skip all of that: raw mem f32 is stored straight from
        the ring per scan block, host does the exact mem>b compare --
        the kernel tail is scan->store, no sign/PE/drain latency.

HBM traffic per core: 13.1MB input + ~0.57MB packed + 1.57MB raw tail.
Loads ride the SP + ACT HWDGE queues at ~10-step granularity (one queue
alone tops out at ~325GB/s, two reach ~425; DMA-completion semaphores
lag data by several us, so only the last small chunk's lag is exposed).
Consts load via the quiet gpsimd SWDGE queue during the preamble; bulk
data must NOT use it (SWDGE descriptor generation stalls everything).
Timeline: ~6.6us fixed preamble, loads visible by ~43.5us, scan tail +
pack tail both land ~50-51.5us, ~7us fixed semaphore-slot teardown.

Sharding: batch dim (64) split 8 ways -> per core [T=100, 8, 4096] =
[128 partitions, 100*256 f32] time-major; whole input persists in SBUF.
"""

import os

import numpy as np

import concourse.bass as bass
import concourse.mybir as mybir
from concourse.bass_utils import run_bass_kernel_spmd

T, B, N = 100, 64, 4096
NCORES = 8
P = 128
BPC = B // NCORES          # batch rows per core
FD = BPC * N // P          # 256 free elems per partition per timestep
F32 = mybir.dt.float32
F8 = mybir.dt.float8e4
I8 = mybir.dt.int8

EPS = np.float32(1e-8)

# DVE scan blocks (custom op validated exact up to 12 steps; 10 used).
# 80-90 is split 5+5 so the packed pipeline's last sign blocks are short.
BLOCKS = [2, 3, 5] + [10] * 7 + [5, 5, 4, 3, 2, 1]
assert sum(BLOCKS) == T
STARTS = [sum(BLOCKS[:i]) for i in range(len(BLOCKS))]
ENDS = [STARTS[i] + BLOCKS[i] for i in range(len(BLOCKS))]
NBL = len(BLOCKS)
NSGN = 12                  # sign blocks 0..11 cover steps 0..90
MRING = 60                 # mem ring length in steps

# input load chunks = DVE blocks (fine granularity keeps the DVE overlapped
# with the load tail; DMA-completion sem updates lag data by several us, so
# only the last small chunk's lag lands on the critical path)
LCH = list(BLOCKS)
LCUM = [sum(LCH[:i + 1]) for i in range(len(LCH))]
NLC = len(LCH)
SP_CH = list(range(0, NLC, 2))     # chunks issued on the SP queue
AC_CH = list(range(1, NLC, 2))     # chunks issued on the ACT queue

# pack geometry: pair = 2 steps (512 cols), tile = 4 pairs -> PSUM [128,512].
# Steps 88..100 skip the pack path entirely: raw mem f32 is stored straight
# from the ring per scan block (gated on DVE progress only), and the host
# does the exact mem>b compare.  The kernel tail is then scan->store.
TRAW = 88
NPAIR = TRAW // 2                  # 44 packed pairs
NTILE = NPAIR // 4                 # 11 tiles
PKW = NTILE * 512                  # packed out cols (i8)

# packed store groups on SP (the final tile 10 is stored by ACT right after
# its own drain -- same-engine ordering, no semaphore, empty queue)
STG = [(0, 4), (4, 8), (8, 10)]

# tail raw-mem stores on SP: (start step, n steps, mrd gate); per-block
# granularity so each store's transfer starts as early as possible and
# the final one is tiny.
TAILMS = [(88, 2, 12), (90, 4, 13), (94, 3, 14), (97, 2, 15), (99, 1, 16)]

_TRACE = bool(os.environ.get("LIF_TRACE"))
LAST_RUN = None  # BassKernelResults of the most recent run (for test.py)


# ---------------------------------------------------------------------------
# Custom fused DVE op registration (runtime-append to concourse.dve_ops.OPS)
# ---------------------------------------------------------------------------
_REGISTERED = {}


def _register_lif_op(ge_mask: bool):
    """LIF_STEP_(LE|GE): out = (Src0 cmp C0) * Src0 * C1 + Src1."""
    name = "LIF_STEP_GE" if ge_mask else "LIF_STEP_LE"
    if name in _REGISTERED:
        return _REGISTERED[name]
    import concourse.dve_ops as dops
    from concourse.dve_spec import Spec, Src0, Src1, C0, C1, lower
    from concourse.dve_uop import DveOpSpec

    if ge_mask:
        body = (Src0 >= C0) * Src0 * C1 + Src1
        ref = lambda in0, in1, s0, s1, imm2: (
            (in0 * (in0 >= s0).astype(np.float32)).astype(np.float32)
            * np.float32(s1) + in1).astype(np.float32)
    else:
        body = (Src0 <= C0) * Src0 * C1 + Src1
        ref = lambda in0, in1, s0, s1, imm2: (
            (in0 * (in0 <= s0).astype(np.float32)).astype(np.float32)
            * np.float32(s1) + in1).astype(np.float32)
    spec = Spec(body=body, reference=ref)

    row = dops._CUSTOM_DVE_ROW_BASE + len(dops.OPS)
    assert row < 0x20, "custom-DVE opcode rows exhausted"
    shas = {}
    for ver in ("v3", "v4"):
        shas[ver] = DveOpSpec(
            name=name, opcode=row, uops=lower(spec, ver=ver), rd1_en=True
        ).sha(ver)
    op = dops.DveOp(name, spec, subdim=False, uops_sha=shas)
    dops.OPS.append(op)
    dops.CUSTOM_DVE_SPECS[name] = spec
    dops._SUB_OPCODE_FOR_NAME[name] = row
    _REGISTERED[name] = op
    return op


# ---------------------------------------------------------------------------
# Threshold boundary (host-side, exact)
# ---------------------------------------------------------------------------
def _predicate(vth: np.float32):
    c = np.float32(vth + EPS)
    assert c != 0.0, "degenerate threshold"
    one = np.float32(1.0)
    if vth > 0:
        pred = lambda m: (np.float32(np.float32(m) / c) - one) > 0
        increasing = True
    else:
        pred = lambda m: (one - np.float32(np.float32(m) / c)) > 0
        increasing = c < 0
    return pred, increasing


def _f32_key(m) -> int:
    i = int(np.frombuffer(np.float32(m).tobytes(), np.uint32)[0])
    return i ^ 0xFFFFFFFF if i & 0x80000000 else i | 0x80000000


def _key_f32(k: int):
    u = (k & 0x7FFFFFFF) if k & 0x80000000 else (k ^ 0xFFFFFFFF)
    return np.frombuffer(np.uint32(u).tobytes(), np.float32)[0]


def spike_boundary(vth: np.float32):
    """Exact f32 boundary b of the spike predicate.
    spk_is_gt: spk = (mem > b), device no-spike mask = (mem is_le b)."""
    with np.errstate(over="ignore"):
        pred, increasing = _predicate(vth)
        lo_k, hi_k = _f32_key(np.float32(-3.4e38)), _f32_key(np.float32(3.4e38))
        if increasing:
            assert not pred(_key_f32(lo_k)) and pred(_key_f32(hi_k))
            while hi_k - lo_k > 1:
                mid = (lo_k + hi_k) // 2
                if pred(_key_f32(mid)):
                    hi_k = mid
                else:
                    lo_k = mid
            b = _key_f32(lo_k)
            assert not pred(b) and pred(_key_f32(lo_k + 1))
            return b, True
        else:
            assert pred(_key_f32(lo_k)) and not pred(_key_f32(hi_k))
            while hi_k - lo_k > 1:
                mid = (lo_k + hi_k) // 2
                if pred(_key_f32(mid)):
                    lo_k = mid
                else:
                    hi_k = mid
            b = _key_f32(hi_k)
            assert not pred(b) and pred(_key_f32(hi_k - 1))
            return b, False


# ---------------------------------------------------------------------------
# Static schedule helpers
# ---------------------------------------------------------------------------
def _sign_block_for_step(s: int) -> int:
    return next(j for j in range(NBL) if STARTS[j] <= s < ENDS[j])


def _tile_gate_block(t: int) -> int:
    # tile t needs steps < min(8(t+1), TRAW) signed
    need = min(8 * (t + 1), TRAW)
    return next(j for j in range(NBL) if ENDS[j] >= need)


def _act_oplist():
    """ACT program order: ("sign", b) and ("drain", t) interleaved.
    Drain t placed after sign block min(gate(t)+1, NSGN-1)."""
    place = {}
    for t in range(NTILE):
        pb = min(_tile_gate_block(t) + 1, NSGN - 1)
        if t in (8, 9):
            # keep the last sign (gates the final PE tile) ahead of these
            pb = NSGN - 1
        place.setdefault(pb, []).append(t)
    ops = []
    for b in range(NSGN):
        ops.append(("sign", b))
        for t in place.get(b, []):
            ops.append(("drain", t))
    assert len(ops) == NSGN + NTILE
    return ops


ACTOPS = _act_oplist()
APSIDX = {op: i + 1 for i, op in enumerate(ACTOPS)}  # aps value when op done


# ---------------------------------------------------------------------------
# Device program
# ---------------------------------------------------------------------------
def build_program(c2: float, d: float, spk_is_gt: bool) -> bass.Bass:
    lif_op = _register_lif_op(ge_mask=not spk_is_gt)
    nc = bass.Bass("TRN2", target_bir_lowering=False, debug=False,
                   enable_asserts=False)
    x_d = nc.dram_tensor("x", [P, T * FD], F32, kind="ExternalInput")
    w_d = nc.dram_tensor("w", [P, 32], F32, kind="ExternalInput")
    nb_d = nc.dram_tensor("nbias", [P, 1], F32, kind="ExternalInput")
    y_d = nc.dram_tensor("y", [P, PKW], I8, kind="ExternalOutput")
    yr_d = nc.dram_tensor("yr", [P, (T - TRAW) * FD], F32, kind="ExternalOutput")

    xb = nc.alloc_sbuf_tensor("xb", [P, T * FD], F32)        # whole input
    mb = nc.alloc_sbuf_tensor("mb", [P, MRING * FD], F32)    # mem ring
    code = nc.alloc_sbuf_tensor("code", [P, T * FD], F8)     # sign codes
    pko = nc.alloc_sbuf_tensor("pko", [P, PKW], I8)          # packed out
    wf = nc.alloc_sbuf_tensor("wf", [P, 32], F32)
    w8 = nc.alloc_sbuf_tensor("w8", [P, 32], F8)
    cst = nc.alloc_sbuf_tensor("const-lif-bias", [P, 1], F32)
    nc.const_aps.aps[(F32, -float(c2))] = cst.ap()

    pk = [nc.alloc_psum_tensor(f"pk{i}", [P, 512], F32) for i in range(8)]

    xc = [nc.alloc_semaphore(f"xc{c}") for c in range(NLC)]
    mrd = nc.alloc_semaphore("mrd")   # DVE block progress
    aps = nc.alloc_semaphore("aps")   # ACT op progress (signs + drains)
    pem = nc.alloc_semaphore("pem")   # PE tile progress
    cbs = nc.alloc_semaphore("cbs")   # sign-bias const loaded
    gpr = nc.alloc_semaphore("gpr")   # weights loaded / converted
    stm = nc.alloc_semaphore("stm")   # store completions

    def xdsl(c):
        lo = (LCUM[c] - LCH[c]) * FD
        return x_d[:, lo:lo + LCH[c] * FD]

    def xsl(c):
        lo = (LCUM[c] - LCH[c]) * FD
        return xb[:, lo:lo + LCH[c] * FD]

    def msl(s0, n=1):
        # steps [s0, s0+n) in ring coords; caller guarantees no wrap
        p = (s0 % MRING) * FD
        assert (s0 % MRING) + n <= MRING, (s0, n)
        return mb[:, p:p + n * FD]

    # mem-ring reuse: DVE block b overwrites ring cells of steps
    # start..end-1 minus MRING -> need those signed (aps threshold)
    def reuse_gate(b):
        last_old = ENDS[b] - 1 - MRING
        if last_old < 0:
            return None
        return APSIDX[("sign", _sign_block_for_step(last_old))]

    with nc.Block() as blk:

        @blk.sync
        def _(sync):
            for c in SP_CH:
                sync.dma_start(xsl(c), xdsl(c)).then_inc(xc[c], 16)
            for t0, t1 in STG[:2]:
                sync.wait_ge(aps, APSIDX[("drain", t1 - 1)])
                sync.dma_start(y_d[:, 512 * t0:512 * t1],
                               pko[:, 512 * t0:512 * t1]).then_inc(stm, 16)
            # tail raw-mem stores, gated on DVE progress only
            for s0, n, g in TAILMS:
                sync.wait_ge(mrd, g)
                sync.dma_start(yr_d[:, (s0 - TRAW) * FD:(s0 - TRAW + n) * FD],
                               msl(s0, n)).then_inc(stm, 16)
            t0, t1 = STG[-1]
            sync.wait_ge(aps, APSIDX[("drain", t1 - 1)])
            sync.dma_start(y_d[:, 512 * t0:512 * t1],
                           pko[:, 512 * t0:512 * t1]).then_inc(stm, 16)
            sync.wait_ge(stm, 16 * (len(STG) + len(TAILMS) + 1))

        @blk.gpsimd
        def _(gp):
            # tiny consts on the quiet gpsimd queue (complete in preamble)
            gp.dma_start(cst[:, :], nb_d[:, :]).then_inc(cbs, 16)
            gp.dma_start(wf[:, :], w_d[:, :]).then_inc(gpr, 16)

        @blk.scalar
        def _(act):
            for c in AC_CH:
                act.dma_start(xsl(c), xdsl(c)).then_inc(xc[c], 16)
            act.wait_ge(cbs, 16)             # sign-bias const loaded
            first = True
            for kind, i in ACTOPS:
                if kind == "sign":
                    b = i
                    s0, K = STARTS[b], BLOCKS[b]
                    if s0 + K > TRAW:
                        # steps >= TRAW ship as raw mem; don't sign them
                        K = TRAW - s0
                    ins = act.activation(
                        code[:, s0 * FD:(s0 + K) * FD],
                        msl(s0, K),
                        mybir.ActivationFunctionType.Sign,
                        bias=-float(c2), scale=1.0,
                    )
                    ins._wait_ge(mrd, b + 1)
                else:
                    t = i
                    ins = act.copy(pko[:, 512 * t:512 * (t + 1)],
                                   pk[t % 8][:, :])
                    ins._wait_ge(pem, t + 1)
                ins.then_inc(aps, 1)
                if first:
                    # weight fp8 convert tucked behind sign block 0
                    first = False
                    cv = act.copy(w8[:, :], wf[:, :])
                    cv._wait_ge(gpr, 16)
                    cv.then_inc(gpr, 1)
            # tile 10 store straight after its drain (same engine, in order)
            act.dma_start(y_d[:, 512 * (NTILE - 1):512 * NTILE],
                          pko[:, 512 * (NTILE - 1):512 * NTILE]
                          ).then_inc(stm, 16)

        @blk.vector
        def _(v):
            for b in range(NBL):
                g = reuse_gate(b)
                if g is not None:
                    v.wait_ge(aps, g)
                s0, K = STARTS[b], BLOCKS[b]
                cb = b
                if b == 0:
                    ins = v.tensor_copy(msl(0), xb[:, 0:FD])
                    ins._wait_ge(xc[0], 16)
                    last = v._custom_dve(
                        lif_op, out=msl(1), in0=msl(0),
                        in1=xb[:, FD:2 * FD], s0=float(c2), s1=float(d))
                elif (s0 - 1) % MRING + K > MRING:
                    # ring wrap between in0 start and out end: stitch 1 step
                    ins = v._custom_dve(
                        lif_op, out=msl(s0), in0=msl(s0 - 1),
                        in1=xb[:, s0 * FD:(s0 + 1) * FD],
                        s0=float(c2), s1=float(d))
                    ins._wait_ge(xc[cb], 16)
                    last = ins
                    if K > 1:
                        last = v._custom_dve(
                            lif_op, out=msl(s0 + 1, K - 1),
                            in0=msl(s0, K - 1),
                            in1=xb[:, (s0 + 1) * FD:(s0 + K) * FD],
                            s0=float(c2), s1=float(d))
                else:
                    last = v._custom_dve(
                        lif_op, out=msl(s0, K), in0=msl(s0 - 1, K),
                        in1=xb[:, s0 * FD:(s0 + K) * FD],
                        s0=float(c2), s1=float(d))
                    last._wait_ge(xc[cb], 16)
                last.then_inc(mrd, 1)

        @blk.tensor
        def _(pe):
            pe.wait_ge(gpr, 17)              # w8 converted
            for t in range(NTILE):
                if t == NTILE - 1:
                    # first half (pairs 40,41 = steps 80..84): sign block 10
                    gb = next(j for j in range(NBL) if ENDS[j] >= 8 * t + 4)
                else:
                    gb = _tile_gate_block(t)
                thr = APSIDX[("sign", gb)]
                if t >= 8:
                    thr = max(thr, APSIDX[("drain", t - 8)])
                pe.wait_ge(aps, thr)
                for m in range(4):
                    q = 4 * t + m
                    if t == NTILE - 1 and m == 2:
                        # pairs 42,43 (steps 84..88) need sign block 11
                        pe.wait_ge(aps, APSIDX[("sign", NSGN - 1)])
                    ins = pe.matmul(
                        out=pk[t % 8][32 * m:32 * (m + 1), :],
                        lhsT=w8[:, :],
                        rhs=code[:, 512 * q:512 * (q + 1)],
                        start=True, stop=True,
                        tile_position=(0, 32 * m),
                    )
                ins.then_inc(pem, 1)

    mybir.codegen_inst_isa_subclasses(nc)
    return nc


_PROGRAM_CACHE: dict = {}


def _pack_weights() -> np.ndarray:
    w = np.zeros((P, 32), dtype=np.float32)
    for p in range(P):
        w[p, p // 4] = 4.0 ** (p % 4)
    return w


def _decode_core(y: np.ndarray, yr: np.ndarray, bnd: np.float32,
                 spk_is_gt: bool) -> np.ndarray:
    """y: [128, NTILE*512] int8 packed (steps 0..TRAW),
    yr: [128, (T-TRAW)*256] f32 raw mem -> spikes [T, 128, 256] bool."""
    # rows: 128 = 4 pair-slots (m) x 32 row-groups (r)
    # cols: NTILE*512 = NTILE tiles (t) x 2 step-offsets (so) x 256 elems (e)
    S = y.reshape(4, 32, NTILE, 2, 256).astype(np.int32)
    target = 1 if spk_is_gt else -1
    sp = np.empty((T, 128, 256), dtype=bool)
    pk = sp[:TRAW].reshape(NTILE, 4, 2, 32, 4, 256)
    for j in range(4):
        r = np.mod(S, 4)
        cj = np.where(r == 3, -1, r)
        # axes (m, r, t, so, e) -> (t, m, so, r, e)
        pk[:, :, :, :, j, :] = (cj == target).transpose(2, 0, 3, 1, 4)
        S = (S - cj) // 4
    raw = yr.reshape(128, T - TRAW, 256)
    cmp = (raw > bnd) if spk_is_gt else (raw < bnd)
    sp[TRAW:] = cmp.transpose(1, 0, 2)
    return sp


def kernel(inpt: np.ndarray, v_th: np.ndarray, v_decay: np.ndarray) -> np.ndarray:
    global LAST_RUN
    x = np.ascontiguousarray(np.asarray(inpt, dtype=np.float32))
    assert x.shape == (T, B, N), x.shape
    vth = np.float32(np.asarray(v_th))
    d = float(np.float32(np.asarray(v_decay)))
    b, spk_is_gt = spike_boundary(vth)

    key = (float(b), d, spk_is_gt)
    if key not in _PROGRAM_CACHE:
        _PROGRAM_CACHE[key] = build_program(float(b), d, spk_is_gt)
    nc = _PROGRAM_CACHE[key]

    in_maps = []
    nbias = np.full((P, 1), -np.float32(b), dtype=np.float32)
    wpack = _pack_weights()
    for k in range(NCORES):
        xk = x[:, k * BPC:(k + 1) * BPC, :].reshape(T, P, FD)
        xk = np.ascontiguousarray(xk.transpose(1, 0, 2)).reshape(P, T * FD)
        in_maps.append({"x": xk, "nbias": nbias, "w": wpack})

    res = run_bass_kernel_spmd(
        nc, in_maps, core_ids=list(range(NCORES)), trace=_TRACE
    )
    LAST_RUN = res

    spikes = np.empty((T, B, N), dtype=np.float32)
    for k in range(NCORES):
        sp = _decode_core(res.results[k]["y"], res.results[k]["yr"],
                          np.float32(b), spk_is_gt)
        spikes[:, k * BPC:(k + 1) * BPC, :] = sp.reshape(T, BPC, N)
    return spikes
